# revision 38
# baseline (speedup 1.0000x reference)
"""Trainium2 Bass kernel for nn_MultiHeadAttention_25031069401563.

Sharding: 8 cores = (batch b in {0,1}) x (kv-head group g in {0..3}).
Each core computes, for its batch and its kv group (4 query heads, 1 kv head):
  Q/K/V projections, QK-RMSNorm (folded), RoPE, causal attention, and the
  partial o_proj against its 512-column slice of Wo.  The host sums the 4
  partial outputs per batch (tensor-parallel all-reduce done on host).

Device algorithm (per core), all matmuls bf16 x bf16 -> fp32 PSUM.  The
work is emitted in 4 groups so projection (phase 1) and attention
(phase 2) interleave: group g = {proj token tiles 4g..4g+3, attention
q-chunk g, o_proj chunk g} — attention for q-chunk g only needs K/V/Q
token tiles 0..4g+3, so PE never stalls on a phase boundary.

Per token tile: q = x @ WqT and kv = x @ [WkT|WvT] accumulated over H
tiles; sum-of-squares via ScalarE Square+accum_out; RoPE applied to the
RAW psum values (rope commutes with the per-token RMS scale), then one
fused multiply applies the 1/rms scale; Q/K head tiles are transposed to
[head_dim, token] layout by DMA XBAR transposes (no PE/copy cost).

Attention per (q-chunk, head): S^T tiles = K^T-tile.T @ Q^T-chunk (two
tiles per 2-bank psum pair -> ONE exp activation, with the dead left part
of diagonal tiles skipped by the matmuls), causal masking via GpSimd
affine_select on E, denominator via DVE quad-sums + GpSimd cross-partition
reduce (no PE work), attn^T accumulated as V-tile.T @ E with emission two
pairs behind the score matmuls so PE never waits on exp; normalize with
reciprocal + partition_broadcast.  Emission is software-pipelined across
groups: group g = {x DMAs for g+1, proj tiles of g, attention chunk g-1
with o_proj chunk g-2 matmul blocks interleaved as PE fillers}.
"""

import functools

import ml_dtypes
import numpy as np

H = 2048
S = 2048
HD = 128
NH = 16
NKV = 4
EPS = 1e-6
ROPE_BASE = 10000.0

P = 128
NT = S // P            # 16 token tiles
NHT = H // P           # 16 hidden tiles
QC = 512               # q-chunk width (free dim of S^T / attn^T tiles)
NQC = S // QC          # 4
NQH = NH // NKV        # 4 query heads per core
NG = NT // NQC         # 4 token tiles per group
NCORES = 8
B = 2
NXB = 8                # x input token-block DMA count

BF16 = ml_dtypes.bfloat16


# ---------------------------------------------------------------- host prep

def _rope_tables():
    inv_freq = 1.0 / (ROPE_BASE ** (np.arange(0, HD, 2, dtype=np.float32) / HD))
    pos = np.arange(S, dtype=np.float32)
    ang = pos[:, None] * inv_freq[None, :]
    emb = np.concatenate([ang, ang], axis=-1)  # [S, HD]
    return np.cos(emb).astype(np.float32), np.sin(emb).astype(np.float32)


def _fold_tables(cos, sin, w):
    """Fold the RMSNorm elementwise weight into the rope tables.

    Device computes: out[i] = x[i]*cosw[i] + x[(i+64)%128]*sinw[i],
    which must equal (w*x)[i]*cos[i] + rotate_half(w*x)[i]*sin[i]."""
    w = w.astype(np.float32)
    cosw = cos * w[None, :]
    w_rot = np.concatenate([w[64:], w[:64]])
    sgn = np.concatenate([-np.ones(64, np.float32), np.ones(64, np.float32)])
    sinw = sin * (w_rot * sgn)[None, :]
    return cosw, sinw


def _core_inputs(hidden_states, Wq, Wk, Wv, Wo, q_norm_w, k_norm_w):
    cos, sin = _rope_tables()
    cosq, sinq = _fold_tables(cos, sin, np.asarray(q_norm_w))
    cosk, sink = _fold_tables(cos, sin, np.asarray(k_norm_w))
    tables = {
        "cosq": np.ascontiguousarray(cosq.astype(BF16)),
        "sinq": np.ascontiguousarray(sinq.astype(BF16)),
        "cosk": np.ascontiguousarray(cosk.astype(BF16)),
        "sink": np.ascontiguousarray(sink.astype(BF16)),
    }
    x = np.asarray(hidden_states, np.float32)
    Wq = np.asarray(Wq, np.float32)
    Wk = np.asarray(Wk, np.float32)
    Wv = np.asarray(Wv, np.float32)
    Wo = np.asarray(Wo, np.float32)

    def pmaj(a):
        """[n*128, F] -> partition-major [128, n, F] (contiguous per partition)."""
        n = a.shape[0] // P
        return np.ascontiguousarray(
            a.reshape(n, P, -1).transpose(1, 0, 2).astype(BF16))

    in_maps = []
    for core in range(NCORES):
        b, g = core // NKV, core % NKV
        wkv = np.concatenate(
            [Wk[HD * g:HD * (g + 1), :].T, Wv[HD * g:HD * (g + 1), :].T], axis=1)
        # x^T in token-block-major order: [NXB, 128p, NHT, blk] so each block's
        # DMA is one contiguous 8 KiB read per partition, arriving in tt order.
        xT = x[b].T.astype(BF16)                       # [H, S]
        blk = S // NXB
        xb = (xT.reshape(NHT, P, NXB, blk)
              .transpose(2, 1, 0, 3))                  # [NXB, p, ht, blk]
        m = {
            "xT": np.ascontiguousarray(xb),
            "wqT": pmaj(Wq[512 * g:512 * (g + 1), :].T),
            "wkvT": pmaj(wkv),
            "wo": pmaj(Wo[:, 512 * g:512 * (g + 1)].T),
            **{k: pmaj(v) for k, v in tables.items()},
        }
        in_maps.append(m)
    return in_maps


# ------------------------------------------------------------- device build

def _emit_body(nc, tc, mybir, bass, res, work, psum):
    """Emit one full forward pass. `res` holds the resident SBUF tiles."""
    f32 = mybir.dt.float32
    bf = mybir.dt.bfloat16
    Alu = mybir.AluOpType
    Act = mybir.ActivationFunctionType

    d = nc.dram_aps  # dict of dram APs, stashed by _build

    # ---- intro DMAs: just enough for the first proj group; the rest are
    # emitted just-in-time inside the group loop so they never sit ahead of
    # the transposes / output writes in the queue.
    blk = S // NXB

    def dma_x(xb):
        tsl = slice(xb * blk, (xb + 1) * blk)
        nc.sync.dma_start(out=res["xT"][:, :, tsl], in_=d["xT"][xb])

    nc.sync.dma_start(out=res["wq"][:, 0:4, :], in_=d["wqT"][:, 0:4, :])
    nc.sync.dma_start(out=res["xT"][:, :, 0:P], in_=d["xT"][0][:, :, 0:P])
    for wq4 in range(1, 4):
        nc.sync.dma_start(out=res["wq"][:, 4 * wq4:4 * wq4 + 4, :],
                          in_=d["wqT"][:, 4 * wq4:4 * wq4 + 4, :])
    nc.sync.dma_start(out=res["wkv"], in_=d["wkvT"])
    nc.sync.dma_start(out=res["xT"][:, :, P:2 * P], in_=d["xT"][0][:, :, P:2 * P])
    dma_x(1)
    for name in ("cosq", "sinq", "cosk", "sink"):
        nc.sync.dma_start(out=res[name], in_=d[name])

    nc.vector.memset(res["eps_q"], EPS)
    nc.vector.memset(res["eps_k"], HD * EPS)

    # Pre-load the one activation-function set covering Square/Ln/Exp/Copy so
    # the insert_act_table_loads pass never needs to switch sets mid-stream.
    from concourse.hw_specs import get_activation_tables
    nle_id = list(get_activation_tables(nc.m.arch)).index(
        "natural_log_exp_and_others")
    nc.scalar.add_instruction(mybir.InstLoadActFuncSet(
        name=nc.get_next_instruction_name(), act_func_set_id=nle_id,
        ins=[], outs=[]))

    def bcast_heads(ap2d, n):
        return bass.AP(tensor=ap2d.tensor, offset=ap2d.offset,
                       ap=[ap2d.ap[0], [0, n], *ap2d.ap[1:]])

    def rot_view(ap, nh):
        """[P, nh, HD] view reading each head's halves swapped."""
        a = ap.ap
        assert a[-1][0] == 1 and a[-1][1] == HD
        head = [] if nh == 1 else [a[-2]]
        return bass.AP(tensor=ap.tensor, offset=ap.offset + 64,
                       ap=[a[0], *head, [-64, 2], [1, 64]])

    # ---------------- phase 1 for one token tile: proj, norms, rope, qT/kT
    def proj_tile(tt):
        ts = slice(tt * P, (tt + 1) * P)
        qp = psum.tile([P, 4 * HD], f32, tag="ps_a")
        kvp = psum.tile([P, 2 * HD], f32, tag="ps_b")
        # tt 0: all q matmuls then all kv, matching the intro DMA arrival
        # order (wq half 0, x0, wq half 1, wkv); later tiles interleave
        if tt == 0:
            order = [("q", range(NHT)), ("kv", range(NHT))]
        else:
            order = [("q", range(0, 8)), ("kv", range(0, 8)),
                     ("q", range(8, 16)), ("kv", range(8, 16))]
        for kind, hts in order:
            dst, w = (qp, res["wq"]) if kind == "q" else (kvp, res["wkv"])
            for ht in hts:
                nc.tensor.matmul(dst, lhsT=res["xT"][:, ht, ts],
                                 rhs=w[:, ht, :],
                                 start=(ht == 0), stop=(ht == NHT - 1))
        kp = kvp[:, 0:HD]
        vp = kvp[:, HD:2 * HD]
        # q-side norm chain first (independent of kvp), then the kvp-gated
        # work, so the Act queue never head-of-line blocks the q chain.
        # 1/rms as exp(-ln(x)/2): Ln/Exp/Square/Copy share one activation
        # function set, so the scalar engine never reloads tables.
        sums = work.tile([P, 5], f32, tag="sums", bufs=2)
        scr = work.tile([P, HD], bf, tag="scr", bufs=2)
        sl = work.tile([P, 5], f32, tag="sl", bufs=2)
        rc = work.tile([P, 5], f32, tag="rc", bufs=2)
        for h in range(NQH):
            nc.scalar.activation(scr, qp[:, h * HD:(h + 1) * HD], Act.Square,
                                 accum_out=sums[:, h:h + 1])
        nc.scalar.activation(sl[:, 0:4], sums[:, 0:4], Act.Ln,
                             scale=1.0 / HD, bias=res["eps_q"])
        nc.scalar.activation(rc[:, 0:4], sl[:, 0:4], Act.Exp, scale=-0.5)
        # V straight to bf16 SBUF (scalar engine)
        nc.scalar.activation(res["v"][:, tt, :], vp, Act.Copy)
        nc.scalar.activation(scr, kp, Act.Square, accum_out=sums[:, 4:5])
        nc.scalar.activation(sl[:, 4:5], sums[:, 4:5], Act.Ln,
                             scale=1.0, bias=res["eps_k"])
        nc.scalar.activation(rc[:, 4:5], sl[:, 4:5], Act.Exp, scale=-0.5)

        # Q: rope the raw psum values, then one fused 1/rms multiply
        qp4 = bass.AP(tensor=qp.tensor, offset=qp.offset,
                      ap=[qp.ap[0], [HD, NQH], [1, HD]])
        t1 = work.tile([P, NQH, HD], bf, tag="t1")
        t2 = work.tile([P, NQH, HD], bf, tag="t2")
        qr0 = work.tile([P, NQH, HD], bf, tag="qr0")
        qr = work.tile([P, NQH, HD], bf, tag="qr")
        nc.vector.tensor_tensor(t1, qp4, bcast_heads(res["cosq"][:, tt, :], NQH),
                                Alu.mult)
        nc.vector.tensor_tensor(t2, rot_view(qp4, NQH),
                                bcast_heads(res["sinq"][:, tt, :], NQH),
                                Alu.mult)
        nc.vector.tensor_tensor(qr0, t1, t2, Alu.add)
        rcb = bass.AP(tensor=rc.tensor, offset=rc.offset,
                      ap=[rc.ap[0], [1, NQH], [0, HD]])
        nc.vector.tensor_tensor(qr, qr0, rcb, Alu.mult)

        # K: rope raw, then scale by 1/rms
        k1 = work.tile([P, HD], bf, tag="k1")
        k2 = work.tile([P, HD], bf, tag="k2")
        kr0 = work.tile([P, HD], bf, tag="kr0")
        kr = work.tile([P, HD], bf, tag="kr")
        nc.vector.tensor_tensor(k1, kp, res["cosk"][:, tt, :], Alu.mult)
        nc.vector.tensor_tensor(k2, rot_view(kp, 1), res["sink"][:, tt, :],
                                Alu.mult)
        nc.vector.tensor_tensor(kr0, k1, k2, Alu.add)
        nc.vector.tensor_scalar_mul(kr, kr0, rc[:, 4:5])

        # -> [hd, token] layout via DMA XBAR transposes (no PE / copy cost)
        for h in range(NQH):
            nc.sync.dma_start_transpose(res["qT"][:, h, ts], qr[:, h, :])
        nc.sync.dma_start_transpose(res["kT"][:, ts], kr)

    # ---------------- phase 2 for one q-chunk: attention + o_proj
    attnT_of = {}

    def attn_chunk(qc, filler=None):
        qs = slice(qc * QC, (qc + 1) * QC)
        attnT = work.tile([P, NQH, QC], bf, tag="attnT")
        attnT_of[qc] = attnT
        nkt = 4 * qc + 4

        def fill():
            if filler is not None:
                next(filler, None)
        for h in range(NQH):
            av = psum.tile([P, QC], f32, tag="ps_a")
            etot = work.tile([P, QC], bf, tag="etot", bufs=2)
            equad = []
            av_pend = []

            def score_pair(p):
                # two score tiles into one 2-bank psum tile -> ONE exp
                # activation over both (halves the Act fixed overhead)
                st2 = psum.tile([P, 2, QC], f32, tag="ps_c", bufs=2)
                e2 = work.tile([P, 2, QC], bf, tag="e", bufs=4)
                for j in range(2):
                    kt = 2 * p + j
                    # diagonal tiles: only q columns >= kt*P survive the
                    # causal mask — skip the dead left part of the matmul.
                    # The exp of the stale (bounded) psum columns is zeroed
                    # by the affine_select below.
                    off = max(0, (kt - 4 * qc) * P)
                    nc.tensor.matmul(st2[:, j, off:],
                                     lhsT=res["kT"][:, kt * P:(kt + 1) * P],
                                     rhs=res["qT"][:, h, qc * QC + off:
                                                   (qc + 1) * QC],
                                     start=True, stop=True)
                nc.scalar.activation(e2, st2, Act.Exp)
                for j in range(2):
                    kt = 2 * p + j
                    if kt >= 4 * qc:  # diagonal: causal-mask the exp'd tile
                        nc.gpsimd.affine_select(
                            out=e2[:, j, :], in_=e2[:, j, :],
                            compare_op=Alu.is_ge, fill=0.0,
                            base=qc * QC - kt * P, pattern=[[1, QC]],
                            channel_multiplier=-1)
                    av_pend.append((kt, e2))
                equad.append(e2)
                if len(equad) == 2:  # nkt is always a multiple of 4
                    s01 = work.tile([P, QC], bf, tag="s01", bufs=2)
                    s23 = work.tile([P, QC], bf, tag="s23", bufs=2)
                    nc.vector.tensor_tensor(s01, equad[0][:, 0, :],
                                            equad[0][:, 1, :], Alu.add)
                    nc.vector.tensor_tensor(s23, equad[1][:, 0, :],
                                            equad[1][:, 1, :], Alu.add)
                    if 2 * p < 4:  # first quad of this head
                        nc.vector.tensor_tensor(etot, s01, s23, Alu.add)
                    else:
                        esq = work.tile([P, QC], bf, tag="esq", bufs=2)
                        nc.vector.tensor_tensor(esq, s01, s23, Alu.add)
                        nc.vector.tensor_tensor(etot, etot, esq, Alu.add)
                    equad.clear()

            def av_step():
                kt, e2 = av_pend.pop(0)
                off = max(0, (kt - 4 * qc) * P)
                nc.tensor.matmul(av[:, off:], lhsT=res["v"][:, kt, :],
                                 rhs=e2[:, kt % 2, off:],
                                 start=(kt == 0), stop=(kt == nkt - 1))

            # emit scores two pairs ahead of av so PE never waits on exp
            np_ = nkt // 2
            score_pair(0)
            score_pair(1)
            for p in range(2, np_):
                score_pair(p)
                av_step()
                av_step()
                if p % 2 == 0:
                    fill()
            for _ in range(4):
                av_step()

            # denominator: cross-partition sum on GpSimd (keeps PE free)
            rcp = work.tile([1, QC], f32, tag="rcp", bufs=2)
            dnr = work.tile([1, QC], f32, tag="dnr", bufs=2)
            nc.gpsimd.tensor_reduce(dnr, etot, mybir.AxisListType.C, Alu.add)
            nc.vector.reciprocal_approx_fast(rcp, dnr)
            bc = work.tile([P, QC], f32, tag="bc")
            nc.gpsimd.partition_broadcast(bc, rcp)
            nc.vector.tensor_tensor(attnT[:, h, :], av, bc, Alu.mult)

    def o_proj_blocks(qc):
        """Generator: one o_proj output block per next() — lets attention
        emission interleave these matmuls into its exp-paced stretches."""
        attnT = attnT_of.pop(qc)
        for t4 in range(QC // P):
            tt = qc * (QC // P) + t4
            for hc in range(H // 512):
                op = psum.tile([P, 512], f32, tag="ps_b")
                for ft in range(NQH):
                    nc.tensor.matmul(
                        op, lhsT=attnT[:, ft, t4 * P:(t4 + 1) * P],
                        rhs=res["wo"][:, ft, hc * 512:(hc + 1) * 512],
                        start=(ft == 0), stop=(ft == NQH - 1))
                ost = work.tile([P, 512], bf, tag="ost")
                # qc >= 2 runs while the epilogue's Act engine is saturated
                # with chunk-3 exps — keep those copies on DVE
                if qc < 2 and (t4 + hc) % 4 == 1:
                    nc.scalar.activation(ost, op, Act.Copy)
                else:
                    nc.vector.tensor_copy(ost, op)
                # last chunk: split the output flush across both hwdge
                # queues so the kernel tail isn't one serial DMA drain
                eng = nc.scalar if qc == NQC - 1 and hc % 2 else nc.sync
                eng.dma_start(
                    out=d["out"][tt * P:(tt + 1) * P, hc * 512:(hc + 1) * 512],
                    in_=ost)
                yield

    def o_proj(qc):
        for _ in o_proj_blocks(qc):
            pass

    # ---------------- software-pipelined emission: attention chunk g-1 runs
    # under group g's projections (so a chunk never waits for the rope /
    # transpose chain of the group's last token tile), and o_proj of chunk
    # g-2 runs under group g too (so it never waits on the softmax
    # denominator chain of its chunk's last head).  Next group's x blocks
    # (and wo) are DMA'd just-in-time.
    for g in range(NQC):
        # next group's x blocks (and wo) first, so they are never stuck in
        # the queue behind this group's rope-gated transposes
        if g < NQC - 1:
            dma_x(2 * g + 2)
            dma_x(2 * g + 3)
        if g == 0:
            nc.sync.dma_start(out=res["wo"], in_=d["wo"])
        for tt in range(NG * g, NG * (g + 1)):
            proj_tile(tt)
        if g > 0:
            # o_proj of chunk g-2 rides along inside attention g-1's
            # exp-paced stretches, filling PE stalls with its matmuls
            fil = o_proj_blocks(g - 2) if g > 1 else None
            attn_chunk(g - 1, filler=fil)
            if fil is not None:
                for _ in fil:
                    pass
    fil = o_proj_blocks(NQC - 2)
    attn_chunk(NQC - 1, filler=fil)
    for _ in fil:
        pass
    o_proj(NQC - 1)


def _build(with_loop=False):
    import concourse.bass as bass
    import concourse.mybir as mybir
    import concourse.tile as tile
    from concourse import bacc

    f32 = mybir.dt.float32
    bf = mybir.dt.bfloat16

    nc = bacc.Bacc("TRN2", target_bir_lowering=False, debug=False)
    d = {}
    d["xT"] = nc.dram_tensor("xT", [NXB, P, NHT, S // NXB], bf,
                             kind="ExternalInput").ap()
    d["wqT"] = nc.dram_tensor("wqT", [P, NHT, 4 * HD], bf,
                              kind="ExternalInput").ap()
    d["wkvT"] = nc.dram_tensor("wkvT", [P, NHT, 2 * HD], bf,
                               kind="ExternalInput").ap()
    d["wo"] = nc.dram_tensor("wo", [P, NQH, H], bf, kind="ExternalInput").ap()
    for name in ("cosq", "sinq", "cosk", "sink"):
        d[name] = nc.dram_tensor(name, [P, NT, HD], bf,
                                 kind="ExternalInput").ap()
    d["out"] = nc.dram_tensor("out", [S, H], bf, kind="ExternalOutput").ap()
    nc.dram_aps = d

    with tile.TileContext(nc) as tc:
        from contextlib import ExitStack
        with ExitStack() as stk:
            const = stk.enter_context(tc.tile_pool(name="const", bufs=1))
            work = stk.enter_context(tc.tile_pool(name="work", bufs=3))
            psum = stk.enter_context(
                tc.tile_pool(name="psum", bufs=2, space="PSUM"))

            shapes = {
                "xT": ([P, NHT, S], bf),
                "wq": ([P, NHT, 4 * HD], bf),
                "wkv": ([P, NHT, 2 * HD], bf),
                "wo": ([P, NQH, H], bf),
                "cosq": ([P, NT, HD], bf),
                "sinq": ([P, NT, HD], bf),
                "cosk": ([P, NT, HD], bf),
                "sink": ([P, NT, HD], bf),
                "qT": ([P, NQH, S], bf),
                "kT": ([P, S], bf),
                "v": ([P, NT, HD], bf),
                "eps_q": ([P, 1], f32),
                "eps_k": ([P, 1], f32),
            }
            res = {k: const.tile(shape, dt, tag=k, name=k)
                   for k, (shape, dt) in shapes.items()}

            if with_loop and with_loop > 1:
                with tc.For_i(0, int(with_loop)) as _i:
                    _emit_body(nc, tc, mybir, bass, res, work, psum)
            else:
                _emit_body(nc, tc, mybir, bass, res, work, psum)

    nc.compile()
    return nc


@functools.lru_cache(maxsize=4)
def _get_nc(with_loop=0):
    """with_loop: 0/1 = plain single-shot body; N>1 = body wrapped in a
    static hardware For_i loop of N iterations (for timing)."""
    return _build(with_loop=with_loop)


# ------------------------------------------------------------------ kernel

def kernel(hidden_states, attention_mask, Wq, Wk, Wv, Wo, q_norm_w, k_norm_w):
    from concourse import bass_utils

    nc = _get_nc(False)
    in_maps = _core_inputs(hidden_states, Wq, Wk, Wv, Wo, q_norm_w, k_norm_w)
    res = bass_utils.run_bass_kernel_spmd(nc, in_maps,
                                          core_ids=list(range(NCORES)))
    out = np.zeros((B, S, H), np.float32)
    for core in range(NCORES):
        out[core // NKV] += np.asarray(res.results[core]["out"], np.float32)
    return out


# revision 39
# speedup vs baseline: 6.6933x; 6.6933x over previous
"""Trainium2 Bass kernel for nn_MultiHeadAttention_25031069401563.

Sharding: 8 cores = (batch b in {0,1}) x (kv-head group g in {0..3}).
Each core computes, for its batch and its kv group (4 query heads, 1 kv head):
  Q/K/V projections, QK-RMSNorm (folded), RoPE, causal attention, and the
  partial o_proj against its 512-column slice of Wo.  The host sums the 4
  partial outputs per batch (tensor-parallel all-reduce done on host).

Device algorithm (per core), all matmuls bf16 x bf16 -> fp32 PSUM.  The
work is emitted in 4 groups so projection (phase 1) and attention
(phase 2) interleave: group g = {proj token tiles 4g..4g+3, attention
q-chunk g, o_proj chunk g} — attention for q-chunk g only needs K/V/Q
token tiles 0..4g+3, so PE never stalls on a phase boundary.

Per token tile: q = x @ WqT and kv = x @ [WkT|WvT] accumulated over H
tiles; sum-of-squares via ScalarE Square+accum_out; RoPE applied to the
RAW psum values (rope commutes with the per-token RMS scale), then one
fused multiply applies the 1/rms scale; Q/K head tiles are transposed to
[head_dim, token] layout by DMA XBAR transposes (no PE/copy cost).

Attention per (q-chunk, head): S^T tiles = K^T-tile.T @ Q^T-chunk (two
tiles per 2-bank psum pair -> ONE exp activation, with the dead left part
of diagonal tiles skipped by the matmuls), causal masking via GpSimd
affine_select on E, denominator via DVE quad-sums + GpSimd cross-partition
reduce (no PE work), attn^T accumulated as V-tile.T @ E with emission two
pairs behind the score matmuls so PE never waits on exp; normalize with
reciprocal + partition_broadcast.  Emission is software-pipelined across
groups: group g = {x DMAs for g+1, proj tiles of g, attention chunk g-1
with o_proj chunk g-2 matmul blocks interleaved as PE fillers}.
"""

import functools

import ml_dtypes
import numpy as np

H = 2048
S = 2048
HD = 128
NH = 16
NKV = 4
EPS = 1e-6
ROPE_BASE = 10000.0

P = 128
NT = S // P            # 16 token tiles
NHT = H // P           # 16 hidden tiles
QC = 512               # q-chunk width (free dim of S^T / attn^T tiles)
NQC = S // QC          # 4
NQH = NH // NKV        # 4 query heads per core
NG = NT // NQC         # 4 token tiles per group
NCORES = 8
B = 2
NXB = 8                # x input token-block DMA count

BF16 = ml_dtypes.bfloat16


# ---------------------------------------------------------------- host prep

def _rope_tables():
    inv_freq = 1.0 / (ROPE_BASE ** (np.arange(0, HD, 2, dtype=np.float32) / HD))
    pos = np.arange(S, dtype=np.float32)
    ang = pos[:, None] * inv_freq[None, :]
    emb = np.concatenate([ang, ang], axis=-1)  # [S, HD]
    return np.cos(emb).astype(np.float32), np.sin(emb).astype(np.float32)


def _fold_tables(cos, sin, w):
    """Fold the RMSNorm elementwise weight into the rope tables.

    Device computes: out[i] = x[i]*cosw[i] + x[(i+64)%128]*sinw[i],
    which must equal (w*x)[i]*cos[i] + rotate_half(w*x)[i]*sin[i]."""
    w = w.astype(np.float32)
    cosw = cos * w[None, :]
    w_rot = np.concatenate([w[64:], w[:64]])
    sgn = np.concatenate([-np.ones(64, np.float32), np.ones(64, np.float32)])
    sinw = sin * (w_rot * sgn)[None, :]
    return cosw, sinw


def _core_inputs(hidden_states, Wq, Wk, Wv, Wo, q_norm_w, k_norm_w):
    cos, sin = _rope_tables()
    cosq, sinq = _fold_tables(cos, sin, np.asarray(q_norm_w))
    cosk, sink = _fold_tables(cos, sin, np.asarray(k_norm_w))
    tables = {
        "cosq": np.ascontiguousarray(cosq.astype(BF16)),
        "sinq": np.ascontiguousarray(sinq.astype(BF16)),
        "cosk": np.ascontiguousarray(cosk.astype(BF16)),
        "sink": np.ascontiguousarray(sink.astype(BF16)),
    }
    x = np.asarray(hidden_states, np.float32)
    Wq = np.asarray(Wq, np.float32)
    Wk = np.asarray(Wk, np.float32)
    Wv = np.asarray(Wv, np.float32)
    Wo = np.asarray(Wo, np.float32)

    def pmaj(a):
        """[n*128, F] -> partition-major [128, n, F] (contiguous per partition)."""
        n = a.shape[0] // P
        return np.ascontiguousarray(
            a.reshape(n, P, -1).transpose(1, 0, 2).astype(BF16))

    in_maps = []
    for core in range(NCORES):
        b, g = core // NKV, core % NKV
        wkv = np.concatenate(
            [Wk[HD * g:HD * (g + 1), :].T, Wv[HD * g:HD * (g + 1), :].T], axis=1)
        # x^T in token-block-major order: [NXB, 128p, NHT, blk] so each block's
        # DMA is one contiguous 8 KiB read per partition, arriving in tt order.
        xT = x[b].T.astype(BF16)                       # [H, S]
        blk = S // NXB
        xb = (xT.reshape(NHT, P, NXB, blk)
              .transpose(2, 1, 0, 3))                  # [NXB, p, ht, blk]
        m = {
            "xT": np.ascontiguousarray(xb),
            "wqT": pmaj(Wq[512 * g:512 * (g + 1), :].T),
            "wkvT": pmaj(wkv),
            "wo": pmaj(Wo[:, 512 * g:512 * (g + 1)].T),
            **{k: pmaj(v) for k, v in tables.items()},
        }
        in_maps.append(m)
    return in_maps


# ------------------------------------------------------------- device build

def _emit_body(nc, tc, mybir, bass, res, work, psum):
    """Emit one full forward pass. `res` holds the resident SBUF tiles."""
    f32 = mybir.dt.float32
    bf = mybir.dt.bfloat16
    Alu = mybir.AluOpType
    Act = mybir.ActivationFunctionType

    d = nc.dram_aps  # dict of dram APs, stashed by _build

    # ---- intro DMAs: just enough for the first proj group; the rest are
    # emitted just-in-time inside the group loop so they never sit ahead of
    # the transposes / output writes in the queue.
    blk = S // NXB

    def dma_x(xb):
        tsl = slice(xb * blk, (xb + 1) * blk)
        nc.sync.dma_start(out=res["xT"][:, :, tsl], in_=d["xT"][xb])

    nc.sync.dma_start(out=res["wq"][:, 0:4, :], in_=d["wqT"][:, 0:4, :])
    nc.sync.dma_start(out=res["xT"][:, :, 0:P], in_=d["xT"][0][:, :, 0:P])
    for wq4 in range(1, 4):
        nc.sync.dma_start(out=res["wq"][:, 4 * wq4:4 * wq4 + 4, :],
                          in_=d["wqT"][:, 4 * wq4:4 * wq4 + 4, :])
    nc.sync.dma_start(out=res["wkv"], in_=d["wkvT"])
    nc.sync.dma_start(out=res["xT"][:, :, P:2 * P], in_=d["xT"][0][:, :, P:2 * P])
    dma_x(1)
    for name in ("cosq", "sinq", "cosk", "sink"):
        nc.sync.dma_start(out=res[name], in_=d[name])

    from concourse.masks import make_identity
    make_identity(nc, res["ident"])
    nc.vector.memset(res["ones"], 1.0)
    nc.vector.memset(res["eps_q"], EPS)
    nc.vector.memset(res["eps_k"], HD * EPS)

    # Pre-load the one activation-function set covering Square/Ln/Exp/Copy so
    # the insert_act_table_loads pass never needs to switch sets mid-stream.
    from concourse.hw_specs import get_activation_tables
    nle_id = list(get_activation_tables(nc.m.arch)).index(
        "natural_log_exp_and_others")
    nc.scalar.add_instruction(mybir.InstLoadActFuncSet(
        name=nc.get_next_instruction_name(), act_func_set_id=nle_id,
        ins=[], outs=[]))

    def bcast_heads(ap2d, n):
        return bass.AP(tensor=ap2d.tensor, offset=ap2d.offset,
                       ap=[ap2d.ap[0], [0, n], *ap2d.ap[1:]])

    def rot_view(ap, nh):
        """[P, nh, HD] view reading each head's halves swapped."""
        a = ap.ap
        assert a[-1][0] == 1 and a[-1][1] == HD
        head = [] if nh == 1 else [a[-2]]
        return bass.AP(tensor=ap.tensor, offset=ap.offset + 64,
                       ap=[a[0], *head, [-64, 2], [1, 64]])

    # ---------------- phase 1 for one token tile: proj, norms, rope, qT/kT
    tp_pend = []

    def tp_flush():
        ptt, pqr, pkr = tp_pend.pop(0)
        pts = slice(ptt * P, (ptt + 1) * P)
        for h in range(NQH):
            tp = psum.tile([P, P], bf, tag="ps_c", bufs=3)
            nc.tensor.transpose(tp, pqr[:, h, :], res["ident"])
            nc.scalar.activation(res["qT"][:, h, pts], tp, Act.Copy)
        tp = psum.tile([P, P], bf, tag="ps_c", bufs=3)
        nc.tensor.transpose(tp, pkr, res["ident"])
        nc.scalar.activation(res["kT"][:, pts], tp, Act.Copy)

    def proj_tile(tt):
        ts = slice(tt * P, (tt + 1) * P)
        qp = psum.tile([P, 4 * HD], f32, tag="ps_a")
        kvp = psum.tile([P, 2 * HD], f32, tag="ps_b")
        # tt 0: all q matmuls then all kv, matching the intro DMA arrival
        # order (wq half 0, x0, wq half 1, wkv); later tiles interleave
        if tt == 0:
            order = [("q", range(NHT)), ("kv", range(NHT))]
        else:
            order = [("q", range(0, 8)), ("kv", range(0, 8)),
                     ("q", range(8, 16)), ("kv", range(8, 16))]
        for kind, hts in order:
            dst, w = (qp, res["wq"]) if kind == "q" else (kvp, res["wkv"])
            for ht in hts:
                nc.tensor.matmul(dst, lhsT=res["xT"][:, ht, ts],
                                 rhs=w[:, ht, :],
                                 start=(ht == 0), stop=(ht == NHT - 1))
        kp = kvp[:, 0:HD]
        vp = kvp[:, HD:2 * HD]
        # q-side norm chain first (independent of kvp), then the kvp-gated
        # work, so the Act queue never head-of-line blocks the q chain.
        # 1/rms as exp(-ln(x)/2): Ln/Exp/Square/Copy share one activation
        # function set, so the scalar engine never reloads tables.
        sums = work.tile([P, 5], f32, tag="sums", bufs=2)
        scr = work.tile([P, HD], bf, tag="scr", bufs=2)
        sl = work.tile([P, 5], f32, tag="sl", bufs=2)
        rc = work.tile([P, 5], f32, tag="rc", bufs=2)
        for h in range(NQH):
            nc.scalar.activation(scr, qp[:, h * HD:(h + 1) * HD], Act.Square,
                                 accum_out=sums[:, h:h + 1])
        nc.scalar.activation(sl[:, 0:4], sums[:, 0:4], Act.Ln,
                             scale=1.0 / HD, bias=res["eps_q"])
        nc.scalar.activation(rc[:, 0:4], sl[:, 0:4], Act.Exp, scale=-0.5)
        # V straight to bf16 SBUF (scalar engine)
        nc.scalar.activation(res["v"][:, tt, :], vp, Act.Copy)
        nc.scalar.activation(scr, kp, Act.Square, accum_out=sums[:, 4:5])
        nc.scalar.activation(sl[:, 4:5], sums[:, 4:5], Act.Ln,
                             scale=1.0, bias=res["eps_k"])
        nc.scalar.activation(rc[:, 4:5], sl[:, 4:5], Act.Exp, scale=-0.5)

        # Q: rope the raw psum values, then one fused 1/rms multiply
        qp4 = bass.AP(tensor=qp.tensor, offset=qp.offset,
                      ap=[qp.ap[0], [HD, NQH], [1, HD]])
        t1 = work.tile([P, NQH, HD], bf, tag="t1")
        t2 = work.tile([P, NQH, HD], bf, tag="t2")
        qr0 = work.tile([P, NQH, HD], bf, tag="qr0")
        qr = work.tile([P, NQH, HD], bf, tag="qr")
        nc.vector.tensor_tensor(t1, qp4, bcast_heads(res["cosq"][:, tt, :], NQH),
                                Alu.mult)
        nc.vector.tensor_tensor(t2, rot_view(qp4, NQH),
                                bcast_heads(res["sinq"][:, tt, :], NQH),
                                Alu.mult)
        nc.vector.tensor_tensor(qr0, t1, t2, Alu.add)
        rcb = bass.AP(tensor=rc.tensor, offset=rc.offset,
                      ap=[rc.ap[0], [1, NQH], [0, HD]])
        nc.vector.tensor_tensor(qr, qr0, rcb, Alu.mult)

        # K: rope raw, then scale by 1/rms
        k1 = work.tile([P, HD], bf, tag="k1")
        k2 = work.tile([P, HD], bf, tag="k2")
        kr0 = work.tile([P, HD], bf, tag="kr0")
        kr = work.tile([P, HD], bf, tag="kr")
        nc.vector.tensor_tensor(k1, kp, res["cosk"][:, tt, :], Alu.mult)
        nc.vector.tensor_tensor(k2, rot_view(kp, 1), res["sink"][:, tt, :],
                                Alu.mult)
        nc.vector.tensor_tensor(kr0, k1, k2, Alu.add)
        nc.vector.tensor_scalar_mul(kr, kr0, rc[:, 4:5])

        # -> [hd, token] layout queued for PE transposes, emitted one tile
        # behind the projections so PE never waits on the rope chain
        tp_pend.append((tt, qr, kr))
        if len(tp_pend) > 1:
            tp_flush()

    # ---------------- phase 2 for one q-chunk: attention + o_proj
    attnT_of = {}

    def attn_chunk(qc, filler=None):
        qs = slice(qc * QC, (qc + 1) * QC)
        attnT = work.tile([P, NQH, QC], bf, tag="attnT")
        attnT_of[qc] = attnT
        nkt = 4 * qc + 4

        def fill():
            if filler is not None:
                next(filler, None)
        for h in range(NQH):
            av = psum.tile([P, QC], f32, tag="ps_a")
            dn = psum.tile([1, QC], f32, tag="ps_dn", bufs=1)
            equad = []
            av_pend = []
            dn_pend = []

            def dn_step():
                esq, qi = dn_pend.pop(0)
                nc.tensor.matmul(dn, lhsT=res["ones"], rhs=esq,
                                 start=(qi == 0), stop=(qi == nkt // 4 - 1))

            def score(kt):
                st = psum.tile([P, QC], f32, tag="ps_c", bufs=3)
                # diagonal tiles: only q columns >= kt*P survive the causal
                # mask — skip the dead left part of the matmul.  The exp of
                # the stale (bounded) psum columns is zeroed by affine_select.
                off = max(0, (kt - 4 * qc) * P)
                nc.tensor.matmul(st[:, off:],
                                 lhsT=res["kT"][:, kt * P:(kt + 1) * P],
                                 rhs=res["qT"][:, h, qc * QC + off:
                                               (qc + 1) * QC],
                                 start=True, stop=True)
                e = work.tile([P, QC], bf, tag="e", bufs=8)
                nc.scalar.activation(e, st, Act.Exp)
                if kt >= 4 * qc:  # diagonal: causal-mask the exp'd tile
                    nc.gpsimd.affine_select(
                        out=e, in_=e, compare_op=Alu.is_ge, fill=0.0,
                        base=qc * QC - kt * P, pattern=[[1, QC]],
                        channel_multiplier=-1)
                equad.append(e)
                if len(equad) == 4:  # nkt is always a multiple of 4
                    s01 = work.tile([P, QC], bf, tag="s01", bufs=2)
                    s23 = work.tile([P, QC], bf, tag="s23", bufs=2)
                    esq = work.tile([P, QC], bf, tag="esq", bufs=2)
                    nc.vector.tensor_tensor(s01, equad[0], equad[1], Alu.add)
                    nc.vector.tensor_tensor(s23, equad[2], equad[3], Alu.add)
                    nc.vector.tensor_tensor(esq, s01, s23, Alu.add)
                    dn_pend.append((esq, kt // 4))
                    equad.clear()
                av_pend.append((kt, e))

            def av_step():
                kt, e = av_pend.pop(0)
                off = max(0, (kt - 4 * qc) * P)
                nc.tensor.matmul(av[:, off:], lhsT=res["v"][:, kt, :],
                                 rhs=e[:, off:],
                                 start=(kt == 0), stop=(kt == nkt - 1))
                # quad's dn matmul one av-step after its last exp so PE
                # doesn't wait on the DVE quad-sum chain
                if dn_pend and kt >= dn_pend[0][1] * 4 + 3:
                    dn_step()

            # emit score(kt+2) before av(kt) so PE never waits on exp
            score(0)
            score(1)
            for kt in range(2, nkt):
                score(kt)
                av_step()
                if kt % 4 == 0:
                    fill()
            av_step()
            av_step()
            while dn_pend:
                dn_step()

            rcp = work.tile([1, QC], f32, tag="rcp", bufs=2)
            dcp = work.tile([1, QC], f32, tag="dcp", bufs=2)
            nc.vector.tensor_copy(dcp, dn)
            nc.vector.reciprocal_approx_fast(rcp, dcp)
            bc = work.tile([P, QC], f32, tag="bc")
            nc.gpsimd.partition_broadcast(bc, rcp)
            nc.vector.tensor_tensor(attnT[:, h, :], av, bc, Alu.mult)

    def o_proj_blocks(qc):
        """Generator: one o_proj output block per next() — lets attention
        emission interleave these matmuls into its exp-paced stretches."""
        attnT = attnT_of.pop(qc)
        for t4 in range(QC // P):
            tt = qc * (QC // P) + t4
            for hc in range(H // 512):
                op = psum.tile([P, 512], f32, tag="ps_b")
                for ft in range(NQH):
                    nc.tensor.matmul(
                        op, lhsT=attnT[:, ft, t4 * P:(t4 + 1) * P],
                        rhs=res["wo"][:, ft, hc * 512:(hc + 1) * 512],
                        start=(ft == 0), stop=(ft == NQH - 1))
                ost = work.tile([P, 512], bf, tag="ost")
                # qc >= 2 runs while the epilogue's Act engine is saturated
                # with chunk-3 exps — keep those copies on DVE
                if qc < 2 and (t4 + hc) % 4 == 1:
                    nc.scalar.activation(ost, op, Act.Copy)
                else:
                    nc.vector.tensor_copy(ost, op)
                # last chunk: split the output flush across both hwdge
                # queues so the kernel tail isn't one serial DMA drain
                eng = nc.scalar if qc == NQC - 1 and hc % 2 else nc.sync
                eng.dma_start(
                    out=d["out"][tt * P:(tt + 1) * P, hc * 512:(hc + 1) * 512],
                    in_=ost)
                yield

    def o_proj(qc):
        for _ in o_proj_blocks(qc):
            pass

    # ---------------- software-pipelined emission: attention chunk g-1 runs
    # under group g's projections (so a chunk never waits for the rope /
    # transpose chain of the group's last token tile), and o_proj of chunk
    # g-2 runs under group g too (so it never waits on the softmax
    # denominator chain of its chunk's last head).  Next group's x blocks
    # (and wo) are DMA'd just-in-time.
    for g in range(NQC):
        # next group's x blocks (and wo) first, so they are never stuck in
        # the queue behind this group's rope-gated transposes
        if g < NQC - 1:
            dma_x(2 * g + 2)
            dma_x(2 * g + 3)
        if g == 0:
            nc.sync.dma_start(out=res["wo"], in_=d["wo"])
        for tt in range(NG * g, NG * (g + 1)):
            proj_tile(tt)
        while tp_pend:
            tp_flush()
        if g > 0:
            # o_proj of chunk g-2 rides along inside attention g-1's
            # exp-paced stretches, filling PE stalls with its matmuls
            fil = o_proj_blocks(g - 2) if g > 1 else None
            attn_chunk(g - 1, filler=fil)
            if fil is not None:
                for _ in fil:
                    pass
    fil = o_proj_blocks(NQC - 2)
    attn_chunk(NQC - 1, filler=fil)
    for _ in fil:
        pass
    o_proj(NQC - 1)


def _build(with_loop=False):
    import concourse.bass as bass
    import concourse.mybir as mybir
    import concourse.tile as tile
    from concourse import bacc

    f32 = mybir.dt.float32
    bf = mybir.dt.bfloat16

    nc = bacc.Bacc("TRN2", target_bir_lowering=False, debug=False)
    d = {}
    d["xT"] = nc.dram_tensor("xT", [NXB, P, NHT, S // NXB], bf,
                             kind="ExternalInput").ap()
    d["wqT"] = nc.dram_tensor("wqT", [P, NHT, 4 * HD], bf,
                              kind="ExternalInput").ap()
    d["wkvT"] = nc.dram_tensor("wkvT", [P, NHT, 2 * HD], bf,
                               kind="ExternalInput").ap()
    d["wo"] = nc.dram_tensor("wo", [P, NQH, H], bf, kind="ExternalInput").ap()
    for name in ("cosq", "sinq", "cosk", "sink"):
        d[name] = nc.dram_tensor(name, [P, NT, HD], bf,
                                 kind="ExternalInput").ap()
    d["out"] = nc.dram_tensor("out", [S, H], bf, kind="ExternalOutput").ap()
    nc.dram_aps = d

    with tile.TileContext(nc) as tc:
        from contextlib import ExitStack
        with ExitStack() as stk:
            const = stk.enter_context(tc.tile_pool(name="const", bufs=1))
            work = stk.enter_context(tc.tile_pool(name="work", bufs=3))
            psum = stk.enter_context(
                tc.tile_pool(name="psum", bufs=2, space="PSUM"))

            shapes = {
                "xT": ([P, NHT, S], bf),
                "wq": ([P, NHT, 4 * HD], bf),
                "wkv": ([P, NHT, 2 * HD], bf),
                "wo": ([P, NQH, H], bf),
                "cosq": ([P, NT, HD], bf),
                "sinq": ([P, NT, HD], bf),
                "cosk": ([P, NT, HD], bf),
                "sink": ([P, NT, HD], bf),
                "qT": ([P, NQH, S], bf),
                "kT": ([P, S], bf),
                "v": ([P, NT, HD], bf),
                "ident": ([P, P], bf),
                "ones": ([P, 1], bf),
                "eps_q": ([P, 1], f32),
                "eps_k": ([P, 1], f32),
            }
            res = {k: const.tile(shape, dt, tag=k, name=k)
                   for k, (shape, dt) in shapes.items()}

            if with_loop and with_loop > 1:
                with tc.For_i(0, int(with_loop)) as _i:
                    _emit_body(nc, tc, mybir, bass, res, work, psum)
            else:
                _emit_body(nc, tc, mybir, bass, res, work, psum)

    nc.compile()
    return nc


@functools.lru_cache(maxsize=4)
def _get_nc(with_loop=0):
    """with_loop: 0/1 = plain single-shot body; N>1 = body wrapped in a
    static hardware For_i loop of N iterations (for timing)."""
    return _build(with_loop=with_loop)


# ------------------------------------------------------------------ kernel

def kernel(hidden_states, attention_mask, Wq, Wk, Wv, Wo, q_norm_w, k_norm_w):
    from concourse import bass_utils

    nc = _get_nc(False)
    in_maps = _core_inputs(hidden_states, Wq, Wk, Wv, Wo, q_norm_w, k_norm_w)
    res = bass_utils.run_bass_kernel_spmd(nc, in_maps,
                                          core_ids=list(range(NCORES)))
    out = np.zeros((B, S, H), np.float32)
    for core in range(NCORES):
        out[core // NKV] += np.asarray(res.results[core]["out"], np.float32)
    return out
